# revision 42
# baseline (speedup 1.0000x reference)
"""Trainium2 Bass kernel for nn_Block_38517266710836.

reference pipeline: channel mixer -> STFT (hann 2048, hop 1024) -> per-frame
recurrence out[f] = (spec[f] + out[f-1]) * transfer -> iSTFT (hann synthesis)
-> overlap-add -> gain -> tanh.

Sharding: 8 cores, data-parallel over (batch, channel-half): core c handles
batch c//2, mixed channels [32*(c%2), +32). Each core receives its batch's
full 64-channel input (the mixer contracts channels) and writes 32 rows.

Per core (single fused pass, phases interleaved for engine overlap):
  F(fc): mixer chunks for this frame-group (PE matmuls + PE transposes),
         forward windowed DFT as chained [128x128] bf16 matmuls (hann folded
         into host-built weights, m-major weight layout so the first DFT can
         start after 2/16 weight DMAs); eviction into the scan-layout spec
         tile is fused with the T-premultiply (out_f = T*out_{f-1} + T*spec_f)
         on DVE/Pool.
  S(fc): frame recurrence via tensor_tensor_scan on DVE, emitted in eighth-
         chain slices between the next fc's DFT passes so it hides under PE
         work (scan(3) hides under phase I).
  I(fc): inverse windowed iDFT (gain folded into weights, j-major weight
         layout loaded in consumption order), overlap-add folded into PSUM
         accumulation, tanh fused into the ScalarE PSUM eviction,
         corner-turn back via PE transposes, DMA out.
"""

import numpy as np

WINDOW = 2048
STEP = 1024
CPD = 64
BATCH = 4
TIME = 65536
FRAMES = 64
NJ = 16              # per-frame time chunks (fwd contraction blocks)
NM = 16              # spectral slot chunks
DSH = 32             # mixed channels per core
GCH = TIME // 128    # 512 global 128-sample chunks
GPAD = GCH + 16      # + zero pad (frame 63 reaches t=66560; extra width so
                     # the forward rhs slice [base, base+2048) stays in-bounds)
FC = 4               # frame chunks for the scan layout
FW = 16              # frames per chunk
CB = 17              # chain block: 1 inject/reset col + 16 frame cols
SPECW = NM * DSH * CB  # 8704 free cols per fc block
SPECH = SPECW // 2     # scan split point (chain-aligned: 256 chains * 17)


def _hann(n):
    return (0.5 - 0.5 * np.cos(2.0 * np.pi * np.arange(n) / n)).astype(np.float64)


def _slot_tables():
    """slot s in [0,2048): s<1024 -> Re[k=s]; s==1024 -> Re[1024] (parked in
    Im[0]'s slot, since Im[0] is identically 0); s>1024 -> Im[k=s-1024]."""
    k_of_slot = np.zeros(2048, np.int64)
    is_im = np.zeros(2048, np.bool_)
    for s in range(2048):
        if s < 1024:
            k_of_slot[s] = s
        elif s == 1024:
            k_of_slot[s] = 1024
        else:
            k_of_slot[s] = s - 1024
            is_im[s] = True
    return k_of_slot, is_im


def build_fwd_weights():
    """[2048 n, 2048 slots]: windowed rfft of one frame, slot layout."""
    n = np.arange(WINDOW, dtype=np.float64)
    w = _hann(WINDOW)
    k_of_slot, is_im = _slot_tables()
    ang = 2.0 * np.pi * np.outer(n, k_of_slot.astype(np.float64)) / WINDOW
    W = np.where(is_im[None, :], -np.sin(ang), np.cos(ang))
    W *= w[:, None]
    return W


def build_inv_weights(gain):
    """[2048 slots, 2048 n]: gain * hann * irfft from slot layout."""
    n = np.arange(WINDOW, dtype=np.float64)
    w = _hann(WINDOW)
    k_of_slot, is_im = _slot_tables()
    ang = 2.0 * np.pi * np.outer(k_of_slot.astype(np.float64), n) / WINDOW
    k = k_of_slot
    re_coef = (2.0 - (k == 0) - (k == 1024))[:, None] / WINDOW * np.cos(ang)
    im_coef = -2.0 / WINDOW * np.sin(ang)
    W = np.where(is_im[:, None], im_coef, re_coef)
    W[1024, :] = np.cos(np.pi * n) / WINDOW
    W *= (gain * w)[None, :]
    return W


def build_t_slots(transfer):
    k_of_slot, _ = _slot_tables()
    return np.asarray(transfer, np.float64)[:, k_of_slot]  # [ch, 2048]


def build_pattern(t_slots_core):
    """T-pattern [128, SPECW]: per (m,d) chain block of CB cols:
    col 0 = 0 (reset/inject), cols 1..16 = T[slot(m,kf), d]."""
    pat = np.zeros((128, SPECW), np.float64)
    for m in range(NM):
        for d in range(DSH):
            base = (m * DSH + d) * CB
            pat[:, base + 1: base + CB] = \
                t_slots_core[d, m * 128:(m + 1) * 128][:, None]
    return pat


def emulate(x, transfer, mixer_matrix, gain, wdtype=np.float32):
    """Numpy emulation of the device math (offline validation)."""
    b, c, t = x.shape
    Wf = build_fwd_weights().astype(wdtype).astype(np.float64)
    Wi = build_inv_weights(float(np.asarray(gain).ravel()[0])).astype(wdtype).astype(np.float64)
    Ts = build_t_slots(transfer)
    y = np.einsum('bct,cd->bdt', np.asarray(x, np.float64),
                  np.asarray(mixer_matrix, np.float64))
    yp = np.pad(y, ((0, 0), (0, 0), (0, STEP)))
    out = np.zeros((b, c, t), np.float64)
    for bi in range(b):
        frames = np.stack([yp[bi, :, f * STEP: f * STEP + WINDOW]
                           for f in range(FRAMES)], 1)
        spec = frames.astype(wdtype).astype(np.float64) @ Wf
        st = np.zeros((c, 2048))
        outs = np.zeros_like(spec)
        for f in range(FRAMES):
            st = (spec[:, f].astype(wdtype).astype(np.float64) + st) * Ts
            outs[:, f] = st
        aud = outs.astype(wdtype).astype(np.float64) @ Wi
        acc = np.zeros((c, t + STEP))
        for f in range(FRAMES):
            acc[:, f * STEP: f * STEP + WINDOW] += aud[:, f]
        out[bi] = np.tanh(acc[:, :t])
    return out.astype(np.float32)


# ---------------------------------------------------------------------------
# Device program
# ---------------------------------------------------------------------------

_CACHED_NC = None


def _build_program():
    import concourse.bacc as bacc
    import concourse.mybir as mybir
    from concourse import tile
    from contextlib import ExitStack

    f32 = mybir.dt.float32
    bf16 = mybir.dt.bfloat16
    Alu = mybir.AluOpType

    nc = bacc.Bacc("TRN2", target_bir_lowering=False, debug=False, num_devices=8)
    xb = nc.dram_tensor("xb", [CPD, TIME], bf16, kind="ExternalInput").ap()
    mixw = nc.dram_tensor("mixw", [CPD, DSH], bf16, kind="ExternalInput").ap()
    # wf: m-major rows [m*128+p, j*128+c]; wi: j-major rows [j*128+p, m*128+c]
    wf = nc.dram_tensor("wf", [NM * 128, NJ * 128], bf16, kind="ExternalInput").ap()
    wi = nc.dram_tensor("wi", [NJ * 128, NM * 128], bf16, kind="ExternalInput").ap()
    patd = nc.dram_tensor("pat", [128, SPECW], bf16, kind="ExternalInput").ap()
    eyed = nc.dram_tensor("eye", [128, 128], f32, kind="ExternalInput").ap()
    eyebd = nc.dram_tensor("eyeb", [128, 128], bf16, kind="ExternalInput").ap()
    # partition-major output: y2[(fl,d), a4*1024 + j*128 + c] =
    # y[d, a4*4096 + fl*1024 + j*128 + c]; host untangles. This keeps each
    # batched store DMA a 3-dim AP with 512B contiguous runs.
    yout = nc.dram_tensor("y", [128, TIME // 4], f32, kind="ExternalOutput").ap()
    import os
    _dump = os.environ.get("K_DUMP", "")
    dbg = nc.dram_tensor("dbg", [128, FC * SPECW], f32,
                         kind="ExternalOutput").ap() if _dump else None

    XCH = 2048           # x streamed in [64, 2048] chunks (16 g-chunks each)

    def scan_half(eng, fc, lo, hi):
        """Emit recurrence scan for cols [lo, hi) of fc block on engine eng."""
        off = fc * SPECW
        eng.tensor_tensor_scan(
            spec[:, off + lo: off + hi],
            pat[:, lo: hi],
            spec[:, off + lo: off + hi],
            0.0, Alu.mult, Alu.add)

    with tile.TileContext(nc) as tc:
        with tc.tile_pool(name="persist", bufs=1) as persist:
            spec = persist.tile([128, FC * SPECW], bf16, tag="spec")
            pat = persist.tile([128, SPECW], bf16, tag="pat")
            # first inverse-weight block, prefetched during phase F so the
            # F->I seam doesn't stall on the wi DMA queue
            wi0 = persist.tile([128, NM * 128], bf16, tag="wi0")

            # ================= phase F (+ interleaved scans) =================
            with ExitStack() as ctxF:
                wp = ctxF.enter_context(tc.tile_pool(name="wfp", bufs=1))
                mx = wp.tile([CPD, DSH], bf16, tag="mx")
                nc.sync.dma_start(out=mx[:], in_=mixw[:])
                eyeb = wp.tile([128, 128], bf16, tag="eyeb")
                nc.sync.dma_start(out=eyeb[:], in_=eyebd[:])
                wf_t = wp.tile([128, NM * NJ * 128], bf16, tag="wf")
                a_t = wp.tile([128, GPAD * DSH], bf16, tag="a")
                nc.vector.memset(a_t[:, GCH * DSH:], 0.0)

                xin = ctxF.enter_context(tc.tile_pool(name="xin", bufs=4))
                ymp = ctxF.enter_context(tc.tile_pool(name="ymp", bufs=3))
                mp = ctxF.enter_context(tc.tile_pool(name="mp", bufs=2, space="PSUM"))
                tp = ctxF.enter_context(tc.tile_pool(name="tp", bufs=2, space="PSUM"))
                sp = ctxF.enter_context(tc.tile_pool(name="sp", bufs=2, space="PSUM"))

                wf_loaded = 0

                def load_wf_m():
                    nonlocal wf_loaded
                    m = wf_loaded
                    nc.sync.dma_start(
                        out=wf_t[:, m * NJ * 128:(m + 1) * NJ * 128],
                        in_=wf[m * 128:(m + 1) * 128, :])
                    wf_loaded += 1

                def emit_inject(fc_s):
                    """Prepare chain col 0 of block fc_s (reset or carry-in)."""
                    if fc_s == 0:
                        z = spec[:][:, 0:SPECW].rearrange(
                            "p (md c) -> p md c", c=CB)
                        nc.vector.memset(z[:, :, 0:1], 0.0)
                    else:
                        src = spec[:][:, (fc_s - 1) * SPECW: fc_s * SPECW] \
                            .rearrange("p (md c) -> p md c", c=CB)[:, :, CB - 1: CB]
                        dst = spec[:][:, fc_s * SPECW: (fc_s + 1) * SPECW] \
                            .rearrange("p (md c) -> p md c", c=CB)[:, :, 0:1]
                        nc.vector.tensor_copy(dst, src)

                def emit_scan_eighth(fc_s, part):
                    q = SPECW // 8
                    scan_half(nc.vector, fc_s, part * q, (part + 1) * q)

                xc_done = 0
                for fc in range(FC):
                    # --- mixer + corner-turn for the chunks this fc needs ---
                    hi = min(8 * fc + 9, TIME // XCH)
                    for xc in range(xc_done, hi):
                        xt = xin.tile([CPD, XCH], bf16, tag="x")
                        nc.sync.dma_start(out=xt[:], in_=xb[:, xc * XCH:(xc + 1) * XCH])
                        pm = mp.tile([128, 512], f32, tag="mix")
                        for q in range(4):
                            nc.tensor.matmul(
                                pm[q * DSH:(q + 1) * DSH, :],
                                mx[:],
                                xt[:, q * 512:(q + 1) * 512],
                                start=True, stop=True,
                                tile_position=(0, q * DSH))
                        ym = ymp.tile([128, 512], bf16, tag="ym")
                        nc.scalar.copy(ym[:], pm[:])
                        # ym[(q,d), tloc]: t = xc*2048 + q*512 + tloc
                        for gq in range(4):  # per 4 g-chunks (one psum turn tile)
                            pt = tp.tile([128, 128], bf16, tag="turn")
                            nc.tensor.transpose(
                                pt[:],
                                ym[:, gq * 128: gq * 128 + 128],
                                eyeb[:])
                            # pt[tfine, (q2, d)] covers g = xc*16 + q2*4 + gq
                            g0 = xc * (XCH // 128)
                            dst = a_t[:][:, g0 * DSH:(g0 + 16) * DSH] \
                                .rearrange("p (q2 gq d) -> p q2 gq d", q2=4, gq=4)[
                                    :, :, gq, :]
                            psrc = pt[:].rearrange("p (q2 d) -> p q2 d", q2=4)
                            if gq % 2 == 0:
                                nc.scalar.copy(dst, psrc)
                            else:
                                nc.vector.tensor_copy(dst, psrc)
                    xc_done = hi
                    # all weight-block writes must be emitted before the DFT
                    # reads them (deps are registered in emission order)
                    while wf_loaded < NM:
                        load_wf_m()
                    if fc == 0:
                        # queue behind the fwd weights: needed ~40us in
                        nc.sync.dma_start(out=pat[:], in_=patd[:])
                        nc.sync.dma_start(out=wi0[:], in_=wi[0:128, :])

                    # --- forward DFT for this fc (16 frames, m in 8 passes);
                    # the previous fc's recurrence scan is emitted in eighths
                    # between qp groups so DVE stays clear of the mixer's
                    # turn copies and of the qp7 psum evictions ---
                    f16 = fc
                    for qp in range(8):
                        if fc > 0 and qp <= 6:
                            emit_scan_eighth(fc - 1, qp)
                            if qp == 0:
                                emit_scan_eighth(fc - 1, 7)
                        ps = sp.tile([128, 1024], f32, tag="sm")
                        for mi in range(2):
                            m = qp * 2 + mi
                            out_ap = ps[:][:, mi * 512:(mi + 1) * 512] \
                                .rearrange("p (d f) -> p f d", f=16)
                            for j in range(NJ):
                                base = (128 * f16 + j) * DSH
                                rhs = a_t[:][:, base: base + 4096] \
                                    .rearrange("p (f q) -> p f q", f=16)[:, :, :DSH]
                                nc.tensor.matmul(
                                    out_ap,
                                    wf_t[:, (m * NJ + j) * 128:(m * NJ + j + 1) * 128],
                                    rhs,
                                    start=(j == 0), stop=(j == NJ - 1))
                        for mi in range(2):
                            m = qp * 2 + mi
                            src = ps[:][:, mi * 512:(mi + 1) * 512] \
                                .rearrange("p (d f) -> p d f", f=16)
                            doff = fc * SPECW + m * DSH * CB
                            dst = spec[:][:, doff: doff + DSH * CB] \
                                .rearrange("p (d c) -> p d c", c=CB)[:, :, 1: 1 + FW]
                            moff = m * DSH * CB
                            patap = pat[:][:, moff: moff + DSH * CB] \
                                .rearrange("p (d c) -> p d c", c=CB)[:, :, 1: 1 + FW]
                            # eviction fused with the T-premultiply; DVE is
                            # mostly scan-only (the scan otherwise blocks
                            # psum eviction), and GPSIMD cannot read PSUM:
                            # copy on Act, multiply in place on Pool. At qp7
                            # DVE is free (its scan eighths end at qp6) and
                            # Act is about to be busy with the next mixer's
                            # ym evictions, so evict fused on DVE there.
                            if qp == 7:
                                nc.vector.tensor_mul(dst, src, patap)
                            else:
                                nc.scalar.copy(dst, src)
                                nc.gpsimd.tensor_mul(dst, dst, patap)
                    # carry-in column for this block's scan (runs during the
                    # next fc's DFT, or under phase I for the last block)
                    emit_inject(fc)

                # scan(3) runs under phase I's first matmuls
                for part in range(8):
                    emit_scan_eighth(FC - 1, part)

            if _dump == "S":
                with tc.tile_pool(name="dbgp", bufs=1) as dp:
                    dt_ = dp.tile([128, FC * SPECW], f32, tag="dbg")
                    nc.vector.tensor_copy(dt_[:], spec[:])
                    nc.sync.dma_start(out=dbg, in_=dt_[:])

            # ================= phase I =================
            with ExitStack() as ctxI:
                wp2 = ctxI.enter_context(tc.tile_pool(name="wip", bufs=1))
                wi_t = wp2.tile([128, NJ * NM * 128], bf16, tag="wi")
                # load in consumption order: (rp, ji) uses j then j+8
                # (j=0 was prefetched into wi0 during phase F)
                for j in [8, 1, 9, 2, 10, 3, 11, 4, 12, 5, 13, 6, 14, 7, 15]:
                    nc.sync.dma_start(
                        out=wi_t[:, j * NM * 128:(j + 1) * NM * 128],
                        in_=wi[j * 128:(j + 1) * 128, :])

                def wi_blk(j, m):
                    if j == 0:
                        return wi0[:, m * 128:(m + 1) * 128]
                    return wi_t[:, (j * NM + m) * 128:(j * NM + m + 1) * 128]
                eyeb2 = wp2.tile([128, 128], bf16, tag="eyeb2")
                nc.sync.dma_start(out=eyeb2[:], in_=eyebd[:])

                op = ctxI.enter_context(tc.tile_pool(name="ola", bufs=4, space="PSUM"))
                t4 = ctxI.enter_context(tc.tile_pool(name="t4", bufs=4, space="PSUM"))
                tout = ctxI.enter_context(tc.tile_pool(name="tout", bufs=4))
                stg = ctxI.enter_context(tc.tile_pool(name="stg", bufs=6))

                yv = yout.rearrange("p (a4 r) -> p a4 r", r=1024)

                for fc in range(FC):
                    for rp in range(4):
                        for ji in range(2):
                            j = rp * 2 + ji
                            ps = op.tile([128, FW * DSH], f32, tag="ola")
                            out_full = ps[:].rearrange("p (f d) -> p d f", f=FW)
                            # set A: frames 16fc+fi, chunk j
                            for m in range(NM):
                                base = fc * SPECW + m * DSH * CB
                                rhs = spec[:][:, base: base + DSH * CB] \
                                    .rearrange("p (d c) -> p d c", c=CB)[:, :, 1: 1 + FW]
                                nc.tensor.matmul(
                                    out_full,
                                    wi_blk(j, m),
                                    rhs, start=(m == 0), stop=False)
                            # set B: frames 16fc+fi-1, chunk j+8. The inject
                            # col 0 holds out[16fc-1] (zero for fc=0), so the
                            # whole set including the fc boundary is one
                            # 16-frame rhs slice [0:FW].
                            blo = 1 if fc == 0 else 0
                            for m in range(NM):
                                base = fc * SPECW + m * DSH * CB
                                rhs = spec[:][:, base: base + DSH * CB] \
                                    .rearrange("p (d c) -> p d c", c=CB)[:, :, blo: FW]
                                nc.tensor.matmul(
                                    out_full[:, :, blo:],
                                    wi_blk(j + 8, m),
                                    rhs, start=False, stop=(m == NM - 1))
                            # tanh eviction in bf16: |tanh| <= 1 so the
                            # rounding is <= 2e-3 absolute, and bf16
                            # transposes run at 1 cycle/row (f32 takes 2)
                            tt = tout.tile([128, FW * DSH], bf16, tag="to")
                            nc.scalar.activation(
                                tt[:], ps[:], mybir.ActivationFunctionType.Tanh)
                            # corner-turn back + store
                            p4 = t4.tile([128, 512], bf16, tag="t4")
                            for r2 in range(4):
                                nc.tensor.transpose(
                                    p4[:, r2 * 128:(r2 + 1) * 128],
                                    tt[:, r2 * 128:(r2 + 1) * 128],
                                    eyeb2[:])
                            st = stg.tile([128, 512], f32, tag="stg")
                            if ji == 0:
                                nc.vector.tensor_copy(st[:], p4[:])
                            else:
                                nc.scalar.copy(st[:], p4[:])
                            # one batched DMA (4 small ones are
                            # descriptor-bound at 500ns each)
                            dst = yv[:, 4 * fc: 4 * fc + 4, j * 128:(j + 1) * 128]
                            nc.sync.dma_start(
                                out=dst,
                                in_=st[:].rearrange("p (r2 c) -> p r2 c", r2=4))
    nc.compile()
    return nc


def _get_nc():
    global _CACHED_NC
    if _CACHED_NC is None:
        _CACHED_NC = _build_program()
    return _CACHED_NC


def kernel(x, transfer, mixer_matrix, gain, _trace=False):
    import ml_dtypes
    from concourse.bass_utils import run_bass_kernel_spmd

    x = np.ascontiguousarray(np.asarray(x, np.float32))
    transfer = np.asarray(transfer, np.float32)
    mixer_matrix = np.asarray(mixer_matrix, np.float32)
    gain = np.asarray(gain, np.float32)

    bf = ml_dtypes.bfloat16
    Wf = build_fwd_weights()
    Wi = build_inv_weights(float(gain.ravel()[0]))
    # device lhsT block (m,j) = Wf[j*128+p, m*128+c] at wf_dram[m*128+p, j*128+c]
    wf_np = Wf.astype(bf).reshape(NJ, 128, NM, 128) \
        .transpose(2, 1, 0, 3).reshape(NM * 128, NJ * 128)
    # device lhsT block (j,m) = Wi[m*128+p, j*128+c] at wi_dram[j*128+p, m*128+c]
    wi_np = Wi.astype(bf).reshape(NM, 128, NJ, 128) \
        .transpose(2, 1, 0, 3).reshape(NJ * 128, NM * 128)
    Ts = build_t_slots(transfer)
    eye = np.eye(128, dtype=np.float32)
    eyeb = np.eye(128, dtype=np.float64).astype(bf)

    in_maps = []
    for c in range(8):
        b, dh = c // 2, c % 2
        mixw = mixer_matrix[:, dh * DSH:(dh + 1) * DSH].astype(bf)
        patc = build_pattern(Ts[dh * DSH:(dh + 1) * DSH]).astype(bf)
        in_maps.append({
            "xb": x[b].astype(bf),
            "mixw": mixw,
            "wf": np.ascontiguousarray(wf_np),
            "wi": np.ascontiguousarray(wi_np),
            "pat": patc,
            "eye": eye,
            "eyeb": eyeb,
        })

    nc = _get_nc()
    res = run_bass_kernel_spmd(nc, in_maps, list(range(8)), trace=_trace)
    out = np.zeros((BATCH, CPD, TIME), np.float32)
    for c in range(8):
        b, dh = c // 2, c % 2
        y2 = np.asarray(res.results[c]["y"]).reshape(4, DSH, 16, 1024)
        out[b, dh * DSH:(dh + 1) * DSH] = \
            y2.transpose(1, 2, 0, 3).reshape(DSH, TIME)
    import os
    if os.environ.get("K_DUMP", ""):
        np.save("/tmp/dbg0.npy", res.results[0]["dbg"])
    if _trace:
        return out, res
    return out


# revision 44
# speedup vs baseline: 1.0054x; 1.0054x over previous
"""Trainium2 Bass kernel for nn_Block_38517266710836.

reference pipeline: channel mixer -> STFT (hann 2048, hop 1024) -> per-frame
recurrence out[f] = (spec[f] + out[f-1]) * transfer -> iSTFT (hann synthesis)
-> overlap-add -> gain -> tanh.

Sharding: 8 cores, data-parallel over (batch, channel-half): core c handles
batch c//2, mixed channels [32*(c%2), +32). Each core receives its batch's
full 64-channel input (the mixer contracts channels) and writes 32 rows.

Per core (single fused pass, phases interleaved for engine overlap):
  F(fc): mixer chunks for this frame-group (PE matmuls + PE transposes),
         forward windowed DFT as chained [128x128] bf16 matmuls (hann folded
         into host-built weights, m-major weight layout so the first DFT can
         start after 2/16 weight DMAs); eviction into the scan-layout spec
         tile is fused with the T-premultiply (out_f = T*out_{f-1} + T*spec_f)
         on DVE/Pool.
  S(fc): frame recurrence via tensor_tensor_scan on DVE, emitted in eighth-
         chain slices between the next fc's DFT passes so it hides under PE
         work (scan(3) hides under phase I).
  I(fc): inverse windowed iDFT (gain folded into weights, j-major weight
         layout loaded in consumption order), overlap-add folded into PSUM
         accumulation, tanh fused into the ScalarE PSUM eviction,
         corner-turn back via PE transposes, DMA out.
"""

import numpy as np

WINDOW = 2048
STEP = 1024
CPD = 64
BATCH = 4
TIME = 65536
FRAMES = 64
NJ = 16              # per-frame time chunks (fwd contraction blocks)
NM = 16              # spectral slot chunks
DSH = 32             # mixed channels per core
GCH = TIME // 128    # 512 global 128-sample chunks
GPAD = GCH + 16      # + zero pad (frame 63 reaches t=66560; extra width so
                     # the forward rhs slice [base, base+2048) stays in-bounds)
FC = 4               # frame chunks for the scan layout
FW = 16              # frames per chunk
CB = 17              # chain block: 1 inject/reset col + 16 frame cols
SPECW = NM * DSH * CB  # 8704 free cols per fc block
SPECH = SPECW // 2     # scan split point (chain-aligned: 256 chains * 17)


def _hann(n):
    return (0.5 - 0.5 * np.cos(2.0 * np.pi * np.arange(n) / n)).astype(np.float64)


def _slot_tables():
    """slot s in [0,2048): s<1024 -> Re[k=s]; s==1024 -> Re[1024] (parked in
    Im[0]'s slot, since Im[0] is identically 0); s>1024 -> Im[k=s-1024]."""
    k_of_slot = np.zeros(2048, np.int64)
    is_im = np.zeros(2048, np.bool_)
    for s in range(2048):
        if s < 1024:
            k_of_slot[s] = s
        elif s == 1024:
            k_of_slot[s] = 1024
        else:
            k_of_slot[s] = s - 1024
            is_im[s] = True
    return k_of_slot, is_im


def build_fwd_weights():
    """[2048 n, 2048 slots]: windowed rfft of one frame, slot layout."""
    n = np.arange(WINDOW, dtype=np.float64)
    w = _hann(WINDOW)
    k_of_slot, is_im = _slot_tables()
    ang = 2.0 * np.pi * np.outer(n, k_of_slot.astype(np.float64)) / WINDOW
    W = np.where(is_im[None, :], -np.sin(ang), np.cos(ang))
    W *= w[:, None]
    return W


def build_inv_weights(gain):
    """[2048 slots, 2048 n]: gain * hann * irfft from slot layout."""
    n = np.arange(WINDOW, dtype=np.float64)
    w = _hann(WINDOW)
    k_of_slot, is_im = _slot_tables()
    ang = 2.0 * np.pi * np.outer(k_of_slot.astype(np.float64), n) / WINDOW
    k = k_of_slot
    re_coef = (2.0 - (k == 0) - (k == 1024))[:, None] / WINDOW * np.cos(ang)
    im_coef = -2.0 / WINDOW * np.sin(ang)
    W = np.where(is_im[:, None], im_coef, re_coef)
    W[1024, :] = np.cos(np.pi * n) / WINDOW
    W *= (gain * w)[None, :]
    return W


def build_t_slots(transfer):
    k_of_slot, _ = _slot_tables()
    return np.asarray(transfer, np.float64)[:, k_of_slot]  # [ch, 2048]


def build_pattern(t_slots_core):
    """T-pattern [128, SPECW]: per (m,d) chain block of CB cols:
    col 0 = 0 (reset/inject), cols 1..16 = T[slot(m,kf), d]."""
    pat = np.zeros((128, SPECW), np.float64)
    for m in range(NM):
        for d in range(DSH):
            base = (m * DSH + d) * CB
            pat[:, base + 1: base + CB] = \
                t_slots_core[d, m * 128:(m + 1) * 128][:, None]
    return pat


def emulate(x, transfer, mixer_matrix, gain, wdtype=np.float32):
    """Numpy emulation of the device math (offline validation)."""
    b, c, t = x.shape
    Wf = build_fwd_weights().astype(wdtype).astype(np.float64)
    Wi = build_inv_weights(float(np.asarray(gain).ravel()[0])).astype(wdtype).astype(np.float64)
    Ts = build_t_slots(transfer)
    y = np.einsum('bct,cd->bdt', np.asarray(x, np.float64),
                  np.asarray(mixer_matrix, np.float64))
    yp = np.pad(y, ((0, 0), (0, 0), (0, STEP)))
    out = np.zeros((b, c, t), np.float64)
    for bi in range(b):
        frames = np.stack([yp[bi, :, f * STEP: f * STEP + WINDOW]
                           for f in range(FRAMES)], 1)
        spec = frames.astype(wdtype).astype(np.float64) @ Wf
        st = np.zeros((c, 2048))
        outs = np.zeros_like(spec)
        for f in range(FRAMES):
            st = (spec[:, f].astype(wdtype).astype(np.float64) + st) * Ts
            outs[:, f] = st
        aud = outs.astype(wdtype).astype(np.float64) @ Wi
        acc = np.zeros((c, t + STEP))
        for f in range(FRAMES):
            acc[:, f * STEP: f * STEP + WINDOW] += aud[:, f]
        out[bi] = np.tanh(acc[:, :t])
    return out.astype(np.float32)


# ---------------------------------------------------------------------------
# Device program
# ---------------------------------------------------------------------------

_CACHED_NC = None


def _build_program():
    import concourse.bacc as bacc
    import concourse.mybir as mybir
    from concourse import tile
    from contextlib import ExitStack

    f32 = mybir.dt.float32
    bf16 = mybir.dt.bfloat16
    Alu = mybir.AluOpType

    nc = bacc.Bacc("TRN2", target_bir_lowering=False, debug=False, num_devices=8)
    xb = nc.dram_tensor("xb", [2 * CPD, TIME // 2], bf16, kind="ExternalInput").ap()
    mixw = nc.dram_tensor("mixw", [2 * CPD, 2 * DSH], bf16, kind="ExternalInput").ap()
    # wf: m-major rows [m*128+p, j*128+c]; wi: j-major rows [j*128+p, m*128+c]
    wf = nc.dram_tensor("wf", [NM * 128, NJ * 128], bf16, kind="ExternalInput").ap()
    wi = nc.dram_tensor("wi", [NJ * 128, NM * 128], bf16, kind="ExternalInput").ap()
    patd = nc.dram_tensor("pat", [128, SPECW], bf16, kind="ExternalInput").ap()
    eyed = nc.dram_tensor("eye", [128, 128], f32, kind="ExternalInput").ap()
    eyebd = nc.dram_tensor("eyeb", [128, 128], bf16, kind="ExternalInput").ap()
    # partition-major output: y2[(fl,d), a4*1024 + j*128 + c] =
    # y[d, a4*4096 + fl*1024 + j*128 + c]; host untangles. This keeps each
    # batched store DMA a 3-dim AP with 512B contiguous runs.
    yout = nc.dram_tensor("y", [128, TIME // 4], f32, kind="ExternalOutput").ap()
    import os
    _dump = os.environ.get("K_DUMP", "")
    dbg = nc.dram_tensor("dbg", [128, FC * SPECW], f32,
                         kind="ExternalOutput").ap() if _dump else None

    XCH = 2048           # x streamed in [64, 2048] chunks (16 g-chunks each)

    def scan_half(eng, fc, lo, hi):
        """Emit recurrence scan for cols [lo, hi) of fc block on engine eng."""
        off = fc * SPECW
        eng.tensor_tensor_scan(
            spec[:, off + lo: off + hi],
            pat[:, lo: hi],
            spec[:, off + lo: off + hi],
            0.0, Alu.mult, Alu.add)

    with tile.TileContext(nc) as tc:
        with tc.tile_pool(name="persist", bufs=1) as persist:
            spec = persist.tile([128, FC * SPECW], bf16, tag="spec")
            pat = persist.tile([128, SPECW], bf16, tag="pat")
            # first inverse-weight block, prefetched during phase F so the
            # F->I seam doesn't stall on the wi DMA queue
            wi0 = persist.tile([128, NM * 128], bf16, tag="wi0")

            # ================= phase F (+ interleaved scans) =================
            with ExitStack() as ctxF:
                wp = ctxF.enter_context(tc.tile_pool(name="wfp", bufs=1))
                mx = wp.tile([2 * CPD, 2 * DSH], bf16, tag="mx")
                nc.sync.dma_start(out=mx[:], in_=mixw[:])
                eyeb = wp.tile([128, 128], bf16, tag="eyeb")
                nc.sync.dma_start(out=eyeb[:], in_=eyebd[:])
                wf_t = wp.tile([128, NM * NJ * 128], bf16, tag="wf")
                a_t = wp.tile([128, GPAD * DSH], bf16, tag="a")
                nc.vector.memset(a_t[:, GCH * DSH:], 0.0)

                xin = ctxF.enter_context(tc.tile_pool(name="xin", bufs=4))
                ymp = ctxF.enter_context(tc.tile_pool(name="ymp", bufs=3))
                mp = ctxF.enter_context(tc.tile_pool(name="mp", bufs=2, space="PSUM"))
                tp = ctxF.enter_context(tc.tile_pool(name="tp", bufs=2, space="PSUM"))
                sp = ctxF.enter_context(tc.tile_pool(name="sp", bufs=2, space="PSUM"))

                wf_loaded = 0

                def load_wf_m():
                    nonlocal wf_loaded
                    m = wf_loaded
                    nc.sync.dma_start(
                        out=wf_t[:, m * NJ * 128:(m + 1) * NJ * 128],
                        in_=wf[m * 128:(m + 1) * 128, :])
                    wf_loaded += 1

                def emit_inject(fc_s):
                    """Prepare chain col 0 of block fc_s (reset or carry-in)."""
                    if fc_s == 0:
                        z = spec[:][:, 0:SPECW].rearrange(
                            "p (md c) -> p md c", c=CB)
                        nc.vector.memset(z[:, :, 0:1], 0.0)
                    else:
                        src = spec[:][:, (fc_s - 1) * SPECW: fc_s * SPECW] \
                            .rearrange("p (md c) -> p md c", c=CB)[:, :, CB - 1: CB]
                        dst = spec[:][:, fc_s * SPECW: (fc_s + 1) * SPECW] \
                            .rearrange("p (md c) -> p md c", c=CB)[:, :, 0:1]
                        nc.vector.tensor_copy(dst, src)

                def emit_scan_eighth(fc_s, part):
                    q = SPECW // 8
                    scan_half(nc.vector, fc_s, part * q, (part + 1) * q)

                xc_done = 0
                for fc in range(FC):
                    # --- mixer + corner-turn for the chunks this fc needs ---
                    hi = min(8 * fc + 9, TIME // XCH)
                    for xc in range(xc_done, hi):
                        # [128=(q2,ch), (qh,tloc)]: two 64-ch blocks stacked
                        # so the mixer contracts the full 128 partitions
                        # (block-diagonal weights), halving its PE passes
                        xt = xin.tile([2 * CPD, XCH // 2], bf16, tag="x")
                        nc.sync.dma_start(
                            out=xt[:],
                            in_=xb[:, xc * 1024:(xc + 1) * 1024])
                        pm = mp.tile([128, 512], f32, tag="mix")
                        for qh in range(2):
                            nc.tensor.matmul(
                                pm[qh * 64:(qh + 1) * 64, :],
                                mx[:],
                                xt[:, qh * 512:(qh + 1) * 512],
                                start=True, stop=True,
                                tile_position=(0, qh * 64))
                        ym = ymp.tile([128, 512], bf16, tag="ym")
                        nc.scalar.copy(ym[:], pm[:])
                        # ym[(q,d), tloc]: t = xc*2048 + q*512 + tloc
                        for gq in range(4):  # per 4 g-chunks (one psum turn tile)
                            pt = tp.tile([128, 128], bf16, tag="turn")
                            nc.tensor.transpose(
                                pt[:],
                                ym[:, gq * 128: gq * 128 + 128],
                                eyeb[:])
                            # pt[tfine, (q2, d)] covers g = xc*16 + q2*4 + gq
                            g0 = xc * (XCH // 128)
                            dst = a_t[:][:, g0 * DSH:(g0 + 16) * DSH] \
                                .rearrange("p (q2 gq d) -> p q2 gq d", q2=4, gq=4)[
                                    :, :, gq, :]
                            psrc = pt[:].rearrange("p (q2 d) -> p q2 d", q2=4)
                            if gq % 2 == 0:
                                nc.scalar.copy(dst, psrc)
                            else:
                                nc.vector.tensor_copy(dst, psrc)
                    xc_done = hi
                    # all weight-block writes must be emitted before the DFT
                    # reads them (deps are registered in emission order)
                    while wf_loaded < NM:
                        load_wf_m()
                    if fc == 0:
                        # queue behind the fwd weights: needed ~40us in
                        nc.sync.dma_start(out=pat[:], in_=patd[:])
                        nc.sync.dma_start(out=wi0[:], in_=wi[0:128, :])

                    # --- forward DFT for this fc (16 frames, m in 8 passes);
                    # the previous fc's recurrence scan is emitted in eighths
                    # between qp groups so DVE stays clear of the mixer's
                    # turn copies and of the qp7 psum evictions ---
                    f16 = fc
                    for qp in range(8):
                        if fc > 0 and qp <= 6:
                            emit_scan_eighth(fc - 1, qp)
                            if qp == 0:
                                emit_scan_eighth(fc - 1, 7)
                        ps = sp.tile([128, 1024], f32, tag="sm")
                        for mi in range(2):
                            m = qp * 2 + mi
                            out_ap = ps[:][:, mi * 512:(mi + 1) * 512] \
                                .rearrange("p (d f) -> p f d", f=16)
                            for j in range(NJ):
                                base = (128 * f16 + j) * DSH
                                rhs = a_t[:][:, base: base + 4096] \
                                    .rearrange("p (f q) -> p f q", f=16)[:, :, :DSH]
                                nc.tensor.matmul(
                                    out_ap,
                                    wf_t[:, (m * NJ + j) * 128:(m * NJ + j + 1) * 128],
                                    rhs,
                                    start=(j == 0), stop=(j == NJ - 1))
                        for mi in range(2):
                            m = qp * 2 + mi
                            src = ps[:][:, mi * 512:(mi + 1) * 512] \
                                .rearrange("p (d f) -> p d f", f=16)
                            doff = fc * SPECW + m * DSH * CB
                            dst = spec[:][:, doff: doff + DSH * CB] \
                                .rearrange("p (d c) -> p d c", c=CB)[:, :, 1: 1 + FW]
                            moff = m * DSH * CB
                            patap = pat[:][:, moff: moff + DSH * CB] \
                                .rearrange("p (d c) -> p d c", c=CB)[:, :, 1: 1 + FW]
                            # eviction fused with the T-premultiply; DVE is
                            # mostly scan-only (the scan otherwise blocks
                            # psum eviction), and GPSIMD cannot read PSUM:
                            # copy on Act, multiply in place on Pool. At qp7
                            # DVE is free (its scan eighths end at qp6) and
                            # Act is about to be busy with the next mixer's
                            # ym evictions, so evict fused on DVE there.
                            if qp == 7:
                                nc.vector.tensor_mul(dst, src, patap)
                            else:
                                nc.scalar.copy(dst, src)
                                nc.gpsimd.tensor_mul(dst, dst, patap)
                    # carry-in column for this block's scan (runs during the
                    # next fc's DFT, or under phase I for the last block)
                    emit_inject(fc)

                # scan(3) runs under phase I's first matmuls
                for part in range(8):
                    emit_scan_eighth(FC - 1, part)

            if _dump == "S":
                with tc.tile_pool(name="dbgp", bufs=1) as dp:
                    dt_ = dp.tile([128, FC * SPECW], f32, tag="dbg")
                    nc.vector.tensor_copy(dt_[:], spec[:])
                    nc.sync.dma_start(out=dbg, in_=dt_[:])

            # ================= phase I =================
            with ExitStack() as ctxI:
                wp2 = ctxI.enter_context(tc.tile_pool(name="wip", bufs=1))
                wi_t = wp2.tile([128, NJ * NM * 128], bf16, tag="wi")
                # load in consumption order: (rp, ji) uses j then j+8
                # (j=0 was prefetched into wi0 during phase F)
                for j in [8, 1, 9, 2, 10, 3, 11, 4, 12, 5, 13, 6, 14, 7, 15]:
                    nc.sync.dma_start(
                        out=wi_t[:, j * NM * 128:(j + 1) * NM * 128],
                        in_=wi[j * 128:(j + 1) * 128, :])

                def wi_blk(j, m):
                    if j == 0:
                        return wi0[:, m * 128:(m + 1) * 128]
                    return wi_t[:, (j * NM + m) * 128:(j * NM + m + 1) * 128]
                eyeb2 = wp2.tile([128, 128], bf16, tag="eyeb2")
                nc.sync.dma_start(out=eyeb2[:], in_=eyebd[:])

                op = ctxI.enter_context(tc.tile_pool(name="ola", bufs=4, space="PSUM"))
                t4 = ctxI.enter_context(tc.tile_pool(name="t4", bufs=4, space="PSUM"))
                tout = ctxI.enter_context(tc.tile_pool(name="tout", bufs=4))
                stg = ctxI.enter_context(tc.tile_pool(name="stg", bufs=6))

                yv = yout.rearrange("p (a4 r) -> p a4 r", r=1024)

                for fc in range(FC):
                    for rp in range(4):
                        for ji in range(2):
                            j = rp * 2 + ji
                            ps = op.tile([128, FW * DSH], f32, tag="ola")
                            out_full = ps[:].rearrange("p (f d) -> p d f", f=FW)
                            # set A: frames 16fc+fi, chunk j
                            for m in range(NM):
                                base = fc * SPECW + m * DSH * CB
                                rhs = spec[:][:, base: base + DSH * CB] \
                                    .rearrange("p (d c) -> p d c", c=CB)[:, :, 1: 1 + FW]
                                nc.tensor.matmul(
                                    out_full,
                                    wi_blk(j, m),
                                    rhs, start=(m == 0), stop=False)
                            # set B: frames 16fc+fi-1, chunk j+8. The inject
                            # col 0 holds out[16fc-1] (zero for fc=0), so the
                            # whole set including the fc boundary is one
                            # 16-frame rhs slice [0:FW].
                            blo = 1 if fc == 0 else 0
                            for m in range(NM):
                                base = fc * SPECW + m * DSH * CB
                                rhs = spec[:][:, base: base + DSH * CB] \
                                    .rearrange("p (d c) -> p d c", c=CB)[:, :, blo: FW]
                                nc.tensor.matmul(
                                    out_full[:, :, blo:],
                                    wi_blk(j + 8, m),
                                    rhs, start=False, stop=(m == NM - 1))
                            # tanh eviction in bf16: |tanh| <= 1 so the
                            # rounding is <= 2e-3 absolute, and bf16
                            # transposes run at 1 cycle/row (f32 takes 2)
                            tt = tout.tile([128, FW * DSH], bf16, tag="to")
                            nc.scalar.activation(
                                tt[:], ps[:], mybir.ActivationFunctionType.Tanh)
                            # corner-turn back + store
                            p4 = t4.tile([128, 512], bf16, tag="t4")
                            for r2 in range(4):
                                nc.tensor.transpose(
                                    p4[:, r2 * 128:(r2 + 1) * 128],
                                    tt[:, r2 * 128:(r2 + 1) * 128],
                                    eyeb2[:])
                            st = stg.tile([128, 512], f32, tag="stg")
                            if ji == 0:
                                nc.vector.tensor_copy(st[:], p4[:])
                            else:
                                nc.scalar.copy(st[:], p4[:])
                            # one batched DMA (4 small ones are
                            # descriptor-bound at 500ns each)
                            dst = yv[:, 4 * fc: 4 * fc + 4, j * 128:(j + 1) * 128]
                            nc.sync.dma_start(
                                out=dst,
                                in_=st[:].rearrange("p (r2 c) -> p r2 c", r2=4))
    nc.compile()
    return nc


def _get_nc():
    global _CACHED_NC
    if _CACHED_NC is None:
        _CACHED_NC = _build_program()
    return _CACHED_NC


def kernel(x, transfer, mixer_matrix, gain, _trace=False):
    import ml_dtypes
    from concourse.bass_utils import run_bass_kernel_spmd

    x = np.ascontiguousarray(np.asarray(x, np.float32))
    transfer = np.asarray(transfer, np.float32)
    mixer_matrix = np.asarray(mixer_matrix, np.float32)
    gain = np.asarray(gain, np.float32)

    bf = ml_dtypes.bfloat16
    Wf = build_fwd_weights()
    Wi = build_inv_weights(float(gain.ravel()[0]))
    # device lhsT block (m,j) = Wf[j*128+p, m*128+c] at wf_dram[m*128+p, j*128+c]
    wf_np = Wf.astype(bf).reshape(NJ, 128, NM, 128) \
        .transpose(2, 1, 0, 3).reshape(NM * 128, NJ * 128)
    # device lhsT block (j,m) = Wi[m*128+p, j*128+c] at wi_dram[j*128+p, m*128+c]
    wi_np = Wi.astype(bf).reshape(NM, 128, NJ, 128) \
        .transpose(2, 1, 0, 3).reshape(NJ * 128, NM * 128)
    Ts = build_t_slots(transfer)
    eye = np.eye(128, dtype=np.float32)
    eyeb = np.eye(128, dtype=np.float64).astype(bf)

    in_maps = []
    for c in range(8):
        b, dh = c // 2, c % 2
        mxh = mixer_matrix[:, dh * DSH:(dh + 1) * DSH]
        mixw = np.zeros((2 * CPD, 2 * DSH), np.float32)
        mixw[:CPD, :DSH] = mxh
        mixw[CPD:, DSH:] = mxh
        mixw = mixw.astype(bf)
        patc = build_pattern(Ts[dh * DSH:(dh + 1) * DSH]).astype(bf)
        in_maps.append({
            "xb": np.ascontiguousarray(
                x[b].reshape(CPD, TIME // 1024, 2, 512)
                .transpose(2, 0, 1, 3).reshape(2 * CPD, TIME // 2)).astype(bf),
            "mixw": mixw,
            "wf": np.ascontiguousarray(wf_np),
            "wi": np.ascontiguousarray(wi_np),
            "pat": patc,
            "eye": eye,
            "eyeb": eyeb,
        })

    nc = _get_nc()
    res = run_bass_kernel_spmd(nc, in_maps, list(range(8)), trace=_trace)
    out = np.zeros((BATCH, CPD, TIME), np.float32)
    for c in range(8):
        b, dh = c // 2, c % 2
        y2 = np.asarray(res.results[c]["y"]).reshape(4, DSH, 16, 1024)
        out[b, dh * DSH:(dh + 1) * DSH] = \
            y2.transpose(1, 2, 0, 3).reshape(DSH, TIME)
    import os
    if os.environ.get("K_DUMP", ""):
        np.save("/tmp/dbg0.npy", res.results[0]["dbg"])
    if _trace:
        return out, res
    return out
